# revision 65
# baseline (speedup 1.0000x reference)
"""Trainium2 Bass kernel for nn_C4ByteTransformer (4-step carry-propagation
softmax table lookup).

Contract: kernel(**inputs) takes FULL inputs (a_emb[4,256], b_emb[4,256],
W1[514,131072], W2_sum[131072,256], W2_carry[131072,2]) and returns the full
[4,256] float32 output.

Math: the tables are the canonical byte-add lookup structure (verified
exactly on host, with a numpy fallback otherwise):
  scores_i[k] = a_emb[i, a] + b_emb[i, b] + carry[c],  k = 512a + 2b + c
  weights = softmax(10*(scores - 2.5));  out_i = weights @ W2_sum;
  carry' = weights @ W2_carry,  W2_sum[k, (a+b+c) & 255] = 1,
  W2_carry[k, a+b+c >= 256] = 1.
Because exp is multiplicative over the separable score, with
EA[a] = exp(10 a_emb[i,a]), EB[b] = exp(10 b_emb[i,b]) and
s = sigmoid(20 carry_1 - 10) (= F1/(F0+F1)):
  out_i[m] = ((1-s) cyc[m] + s cyc[(m-1) mod 256]) / (ZA ZB)
  carry'_1 = (U + V s) / (ZA ZB)
where cyc = 256-point cyclic convolution of EA and EB,
U = sum_{a+b>=256} EA[a]EB[b], V = sum_{a+b=255} EA[a]EB[b].
The 131072-entry table never has to be touched.

V2 latency structure (the kernel is launch-overhead dominated; ~13.1us of
the exec time is fixed preamble/DMA-latency/teardown measured with a
trivial kernel):
 - The carry recursion is rewritten in tanh form:
     t_{i+1} = tanh(5 vz_i t_i + (10 uz_i + 5 vz_i - 5)),  s = (1+t)/2,
   which the ACT engine evaluates as ONE Tanh per step with per-partition
   scale/bias APs. Tanh lives in the same activation-function set as Exp
   (exp_and_others), so no table swap and no DVE round-trips: the whole
   chain is 3 back-to-back ACT ops.
 - U/V/Z sums: one [128x128] triangular matmul (tri rides the pk DMA as
   f32, bitcast to f32r) gives within-half suffix sums for both halves at
   once; element products and folds are split across DVE and GpSimd; one
   ones-lhsT matmul pair reduces partitions and broadcasts to partitions
   0-3.
 - Final combine: out = zsel*(cyc + ssel*(rot(cyc)-cyc)). d = rot-cyc and
   the zsel pre-scales run on DVE while the chain finishes, leaving a
   single [4,256] op after the step-select.
 - DMA: pkA (small, gates everything) on the sync queue, pkB (a8m+tri) on
   the tensor queue, the four Hankel windows split over gpsimd+vector
   queues. Constants (ones, diag mask) ride pkA; one activation-table
   load; 5 input DMA instructions total.
"""

import os

import numpy as np

NSTEP = 4
D = 256
NE = 131072

_CACHE = {}

LAST_EXEC_TIME_NS = None
LAST_RESULT = None

T0 = float(np.tanh(-5.0))  # chain state for step 0 (s0 = sigmoid(-10))


def _build_nc():
    import concourse.bacc as bacc
    import concourse.mybir as mybir
    import concourse.tile as tile

    f32 = mybir.dt.float32
    f32r = mybir.dt.float32r
    f16 = mybir.dt.float16
    mult = mybir.AluOpType.mult
    add = mybir.AluOpType.add
    subtract = mybir.AluOpType.subtract
    Exp = mybir.ActivationFunctionType.Exp
    Tanh = mybir.ActivationFunctionType.Tanh

    nc = bacc.Bacc("TRN2", target_bir_lowering=False, debug=False,
                   num_devices=1)

    # pkA [128, 57]: b8 (0:8, [bh,i]), a8 (8:16, [ah,i]),
    # a8m (16:48, [ah, i, i'] step-masked), ones4 (48:52),
    # mskZ = diag(1.0) rows 0-3 (52:56), plus a ones column (56) so the
    # ssel accumulate picks up its +0.5 offset for free. Cols 0:48 are
    # exp'd by ONE ACT into f16 (exp(10x) <= e^10 fits; masked -5 lanes
    # underflow to 0).
    pka = nc.dram_tensor("pka", [128, 57], f32, kind="ExternalInput")
    # pkB: tri [128, 128] in f16 (0/1 exact) for a 1-cycle/row suffix
    # matmul with the f16 exp tile as rhs.
    pkb = nc.dram_tensor("pkb", [128, 128], f16, kind="ExternalInput")
    # fp16 halves the window DMA packet time; exp(10b) quantization error
    # from fp16 b-values is ~2.4e-3 worst-case, well inside tolerance.
    # Shaped [.., 2, 128] so the ah1 matmul can take the half-rotated
    # period as ONE reversed-block rhs view.
    bwin = nc.dram_tensor("bwin", [128, NSTEP, 2, 128], f16,
                          kind="ExternalInput")
    # fp16 output: DVE 2-byte ops run in 2x mode and the values are
    # normalized (~[0,1]) before quantization; host casts back to f32.
    out = nc.dram_tensor("out", [NSTEP, D], f16, kind="ExternalOutput")

    with tile.TileContext(nc) as tc:
        with (
            tc.tile_pool(name="sb", bufs=1) as sb,
            tc.tile_pool(name="small", bufs=1) as small,
            tc.tile_pool(name="psA", bufs=1, space="PSUM") as psA,
            tc.tile_pool(name="psC", bufs=1, space="PSUM") as psC,
            tc.tile_pool(name="psD", bufs=1, space="PSUM") as psD,
            tc.tile_pool(name="psW", bufs=1, space="PSUM") as psW,
        ):
            # PE DVFS warm-up: the kernel is too short for the PE clock to
            # ramp out of pstate-low (measured 1.54ns/col on the convs).
            # Four dummy matmuls on memset data fill the otherwise-idle
            # 7.5-9.1us pre-input window to coax the clock up before the
            # real matmul stream begins. Results are never read.
            wsrc = sb.tile([128, 256], f16)
            nc.vector.memset(wsrc[:], 1.0)
            warm_ps = psW.tile([4, 256], f32)
            for _ in range(4):
                nc.tensor.matmul(warm_ps[:], lhsT=wsrc[:, 0:4],
                                 rhs=wsrc[:], start=True, stop=True)
            # ---- input DMAs, spread across queues ----
            pka_sb = sb.tile([128, 57], f32)
            nc.sync.dma_start(pka_sb[:], pka[:])
            pkb_sb = sb.tile([128, 128], f16)
            nc.scalar.dma_start(pkb_sb[:], pkb[:])
            bwin_sb = sb.tile([128, NSTEP, 2, 128], f16)
            # windows 0/1 on the gpsimd queue, 2/3 on sync behind pkA:
            # win0 keeps its packet-bandwidth head start (it gates the
            # serial exp train), and two queues halve descriptor time.
            nc.gpsimd.dma_start(bwin_sb[:, 0:1, :, :], bwin[:, 0:1, :, :])
            nc.sync.dma_start(bwin_sb[:, 2:3, :, :], bwin[:, 2:3, :, :])
            nc.gpsimd.dma_start(bwin_sb[:, 1:2, :, :], bwin[:, 1:2, :, :])
            nc.sync.dma_start(bwin_sb[:, 3:4, :, :], bwin[:, 3:4, :, :])

            # ---- one exp for b8+a8+a8m (ACT, exp_and_others table) ----
            # f16 out: PE runs 16-bit matmuls at 1 cycle/row (f32r needs 4
            # at these sizes); exp(10*a|b) <= e^10 fits fp16, the -5-masked
            # lanes underflow to exactly 0, and products reach e^20 only in
            # f32 outputs/PSUM.
            epk = sb.tile([128, 48], f16)
            nc.scalar.activation(epk[:], pka_sb[:, 0:48], Exp, scale=10.0)
            ewin = sb.tile([128, NSTEP, 2, 128], f16)
            for i in range(NSTEP):
                nc.scalar.activation(ewin[:, i:i + 1, :, :],
                                     bwin_sb[:, i:i + 1, :, :], Exp,
                                     scale=10.0)

            def ebv(bh):  # exp(b8)[:, bh, :] (f16)
                return epk[:, 4 * bh:4 * bh + 4]

            def eav(ah):  # exp(a8)[:, ah, :] (f16)
                return epk[:, 8 + 4 * ah:12 + 4 * ah]

            def eamv(ah, i):  # masked exp(a8) lhsT block, f16
                o = 16 + 16 * ah + 4 * i
                return epk[:, o:o + 4]

            ones4 = pka_sb[:, 48:52]
            mskZ = pka_sb[0:4, 52:56]
            mskZx = pka_sb[0:4, 52:57]
            ones16 = sb.tile([128, 4], f16)
            nc.vector.memset(ones16[:], 1.0)
            triv = pkb_sb[:]

            # ---- suffix sums, both halves in one matmul ----
            # suf[p, (bh,i)] = sum_{q>p} EB_i[128*bh + q]
            suf_ps = psA.tile([128, 2, NSTEP], f32)
            nc.tensor.matmul(suf_ps[:].opt(), lhsT=triv, rhs=epk[:, 0:8],
                             start=True, stop=True)

            # ---- element products (all DVE; v-products first, they only
            # need epka while the u-products wait on the suffix matmul) ----
            # scr: (u1, v1, u2, v2) so the fold can add halves [0:2]+[2:4].
            scr = sb.tile([128, 4, NSTEP], f32)
            nc.vector.tensor_tensor(out=scr[:, 1, :], in0=eav(0),
                                    in1=ebv(1), op=mult)
            nc.vector.tensor_tensor(out=scr[:, 3, :], in0=eav(1),
                                    in1=ebv(0), op=mult)
            nc.vector.tensor_tensor(out=scr[:, 0, :], in0=eav(0),
                                    in1=suf_ps[:, 1, :], op=mult)
            nc.vector.tensor_tensor(out=scr[:, 2, :], in0=eav(1),
                                    in1=suf_ps[:, 0, :], op=mult)

            # ---- partition reduction + broadcast to partitions 0-3 ----
            # red[p, 0, h, k, i]: (h, k) = (u1, v1 | u2, v2)
            # red[p, 1, k, h, i]: (k, h) = (zb0, zb1 | za0, za1)
            red_ps = psC.tile([NSTEP, 2, 2, 2, NSTEP], f32)
            nc.tensor.matmul(red_ps[:, 0, :, :, :].opt(), lhsT=ones4,
                             rhs=scr[:].opt(), start=True, stop=True)
            red_mm2 = nc.tensor.matmul(red_ps[:, 1, :, :, :].opt(),
                                       lhsT=ones16[:], rhs=epk[:, 0:16],
                                       start=True, stop=True)

            # ---- folds (PSUM allows only one PSUM operand per op: copy
            # the 4x32 reduction block to SBUF once, fold from there) ----
            red_sb = small.tile([NSTEP, 2, 2, 2, NSTEP], f32)
            nc.vector.tensor_copy(out=red_sb[:], in_=red_ps[:])
            # sums1 = (U', V); U = U' + ZA1*ZB1 (cross-half correction).
            sums1 = small.tile([NSTEP, 2, NSTEP], f32)
            nc.vector.tensor_tensor(out=sums1[:], in0=red_sb[:, 0, 0, :, :],
                                    in1=red_sb[:, 0, 1, :, :], op=add)
            V5 = small.tile([NSTEP, NSTEP], f32)
            nc.vector.tensor_scalar(out=V5[:], in0=sums1[:, 1, :],
                                    scalar1=5.0, scalar2=None, op0=mult)
            tzz = small.tile([NSTEP, NSTEP], f32)
            nc.gpsimd.tensor_tensor(out=tzz[:], in0=red_sb[:, 1, 0, 1, :],
                                    in1=red_sb[:, 1, 1, 1, :], op=mult)
            U = small.tile([NSTEP, NSTEP], f32)
            nc.vector.tensor_tensor(out=U[:], in0=sums1[:, 0, :], in1=tzz[:],
                                    op=add)
            # W1 = 10U + 5V (DVE) while GpSimd folds Z = (ZB0+ZB1)(ZA0+ZA1)
            W1 = small.tile([NSTEP, NSTEP], f32)
            nc.vector.scalar_tensor_tensor(out=W1[:], in0=U[:], scalar=10.0,
                                           in1=V5[:], op0=mult, op1=add)
            sums2 = small.tile([NSTEP, 2, NSTEP], f32)
            nc.gpsimd.tensor_tensor(out=sums2[:], in0=red_sb[:, 1, :, 0, :],
                                    in1=red_sb[:, 1, :, 1, :], op=add)
            Z = small.tile([NSTEP, NSTEP], f32)
            nc.gpsimd.tensor_tensor(out=Z[:], in0=sums2[:, 0, :],
                                    in1=sums2[:, 1, :], op=mult)

            zbi = small.tile([NSTEP, NSTEP], f32)
            nc.vector.reciprocal(zbi[:], Z[:])

            # ---- chain scale/bias ----
            # scale_i = 5 V_i zbi_i;  bias_i = (10U + 5V)_i zbi_i - 5
            bW = small.tile([NSTEP, NSTEP], f32)
            nc.vector.tensor_tensor(out=bW[:], in0=W1[:], in1=zbi[:],
                                    op=mult)
            scale = small.tile([NSTEP, NSTEP], f32)
            nc.vector.tensor_tensor(out=scale[:], in0=V5[:], in1=zbi[:],
                                    op=mult)
            bias = small.tile([NSTEP, NSTEP], f32)
            bias_ts = nc.vector.tensor_scalar(out=bias[:], in0=bW[:],
                                              scalar1=-5.0, scalar2=None,
                                              op0=add)

            # zsel[p] = zbi[p, p]: fused mask-mult + free-reduce in one STT.
            # Pinned after bias so the scheduler can't slot it into the
            # recip->bias stretch of the chain-critical DVE queue.
            zm = small.tile([NSTEP, NSTEP], f32)
            zsel = small.tile([NSTEP, 1], f32)
            zsel_stt = nc.vector.scalar_tensor_tensor(
                out=zm[:], in0=zbi[:], scalar=1.0, in1=mskZ, op0=mult,
                op1=mult, accum_out=zsel[:])
            tile.add_dep_helper(zsel_stt.ins, bias_ts.ins, False,
                                "chain bias before zsel")

            # ---- carry chain: 3 back-to-back Tanh ACTs ----
            # T col 4 = 1.0 feeds the +0.5 into the ssel accumulate.
            T = small.tile([NSTEP, NSTEP + 1], f32)
            nc.vector.memset(T[:, 0:1], T0)
            nc.vector.memset(T[:, 4:5], 1.0)
            for i in range(NSTEP - 1):
                nc.scalar.activation(T[:, i + 1:i + 2], T[:, i:i + 1], Tanh,
                                     bias=bias[:, i:i + 1],
                                     scale=scale[:, i:i + 1])

            # ---- convolutions: 12 matmuls accumulate into prt[i, m] ----
            # (three split MMs per step pipeline better on the PE than two
            # fused 256-col ones)
            prt = psD.tile([NSTEP, 256], f32)
            for i in range(NSTEP):
                cmm = nc.tensor.matmul(prt[:], lhsT=eamv(0, i),
                                       rhs=ewin[:, i, :, :].opt(),
                                       start=(i == 0), stop=False)
                if i == 0:
                    # keep the bias-path reduction matmuls ahead of the
                    # convolutions on the PE queue
                    tile.add_dep_helper(cmm.ins, red_mm2.ins, False,
                                        "red MMs before convs")
                nc.tensor.matmul(prt[:, 0:128], lhsT=eamv(1, i),
                                 rhs=ewin[:, i, 1, :], start=False,
                                 stop=False)
                nc.tensor.matmul(prt[:, 128:256], lhsT=eamv(1, i),
                                 rhs=ewin[:, i, 0, :], start=False,
                                 stop=(i == NSTEP - 1))

            # ---- combine: out = zsel*cyc + ssel*(zsel*rot(cyc)-zsel*cyc)
            # fp16 pre2/dz/comb: values are normalized by zsel (~[0,1]),
            # and all-2-byte SBUF operands put the DVE in its 2x mode.
            # pre2 = [zsel*cyc[255] | zsel*cyc], so rot(cyc)*zsel is its
            # 0:256 view and dz is ONE self-shifted subtract (wrap incl.)
            pre2 = sb.tile([NSTEP, 257], f16)
            nc.vector.tensor_scalar(out=pre2[:, 0:1], in0=prt[:, 255:256],
                                    scalar1=zsel[:], scalar2=None, op0=mult)
            nc.vector.tensor_scalar(out=pre2[:, 1:257], in0=prt[:],
                                    scalar1=zsel[:], scalar2=None, op0=mult)
            dz = sb.tile([NSTEP, 256], f16)
            dz_tt = nc.vector.tensor_tensor(out=dz[:], in0=pre2[:, 0:256],
                                            in1=pre2[:, 1:257], op=subtract)
            # ssel[p] = (1 + T[p, p]) / 2 in ONE fused accumulate:
            # sum((0.5*[T|1]) * [diag|1]) = 0.5*T[p,p] + 0.5. Pinned after
            # dz so the scheduler can't hoist the chain-gated select ahead
            # of the big pre2/dz ops on the DVE queue.
            tm = small.tile([NSTEP, NSTEP + 1], f32)
            ssel = small.tile([NSTEP, 1], f32)
            ssel_stt = nc.vector.scalar_tensor_tensor(
                out=tm[:], in0=T[:], scalar=0.5, in1=mskZx, op0=mult,
                op1=mult, accum_out=ssel[:])
            tile.add_dep_helper(ssel_stt.ins, dz_tt.ins, False,
                                "big combine ops before chain select")
            comb = sb.tile([NSTEP, D], f16)
            nc.vector.scalar_tensor_tensor(out=comb[:], in0=dz[:],
                                           scalar=ssel[:],
                                           in1=pre2[:, 1:257],
                                           op0=mult, op1=add)
            nc.sync.dma_start(out[:], comb[:])

    nc.compile()
    return nc


def _structure_ok(W1, W2_sum, W2_carry):
    """Exact check that the tables are the canonical byte-add structure."""
    k = np.arange(NE)
    a = k >> 9
    b = (k >> 1) & 255
    c = k & 1
    total = a + b + c
    if W1.shape != (514, NE) or W2_sum.shape != (NE, D):
        return False
    if W2_carry.shape != (NE, 2):
        return False
    if not (W1[a, k] == 1.0).all():
        return False
    if not (W1[256 + b, k] == 1.0).all():
        return False
    if not (W1[512 + c, k] == 1.0).all():
        return False
    if np.abs(W1).sum(dtype=np.float64) != 3.0 * NE:
        return False
    if not (W2_sum[k, total & 255] == 1.0).all():
        return False
    if np.abs(W2_sum).sum(dtype=np.float64) != float(NE):
        return False
    if not (W2_carry[k, (total >= 256).astype(np.int64)] == 1.0).all():
        return False
    if np.abs(W2_carry).sum(dtype=np.float64) != float(NE):
        return False
    return True


def _numpy_fallback(a_emb, b_emb, W1, W2_sum, W2_carry):
    carry = np.zeros(2, dtype=np.float64)
    carry[0] = 1.0
    outs = []
    W1 = W1.astype(np.float64)
    for i in range(NSTEP):
        x = np.concatenate([a_emb[i], b_emb[i], carry]).astype(np.float64)
        scores = x @ W1
        z = (scores - 2.5) * 10.0
        z -= z.max()
        w = np.exp(z)
        w /= w.sum()
        outs.append(w @ W2_sum.astype(np.float64))
        carry = w @ W2_carry.astype(np.float64)
    return np.stack(outs).astype(np.float32)


def _prep_inputs(a_emb, b_emb):
    p = np.arange(128)
    # bwin[j, i, x] = b_emb[i, (j + x + 129) mod 256], one cyclic period
    b_ext = np.take(b_emb, (np.arange(383) + 129) % 256, axis=1)
    bwin = np.ascontiguousarray(
        np.lib.stride_tricks.sliding_window_view(b_ext, 256, axis=1)
        .transpose(1, 0, 2)
    ).astype(np.float16).reshape(128, NSTEP, 2, 128)
    # a8[p, ah, i] = a_emb[i, 128 ah + 127 - p]
    a_r = a_emb[:, ::-1]
    a8 = np.ascontiguousarray(
        a_r.reshape(NSTEP, 2, 128)[:, ::-1, :].transpose(2, 1, 0)
    ).astype(np.float32)
    # a8m: step-masked copy (off-step columns -5 -> exp(10x) ~ 2e-22)
    a8m = np.full((128, 2, NSTEP, NSTEP), -5.0, dtype=np.float32)
    for i in range(NSTEP):
        a8m[:, :, i, i] = a8[:, :, i]
    # b8[p, bh, i] = b_emb[i, 128 bh + p]
    b8 = np.ascontiguousarray(
        b_emb.reshape(NSTEP, 2, 128).transpose(2, 1, 0)
    ).astype(np.float32)
    ones4 = np.ones((128, 4), dtype=np.float32)
    mskZ = np.zeros((128, 5), dtype=np.float32)
    mskZ[np.arange(4), np.arange(4)] = 1.0
    mskZ[:, 4] = 1.0  # +0.5 offset column for the ssel accumulate
    pka = np.concatenate(
        [b8.reshape(128, 8), a8.reshape(128, 8), a8m.reshape(128, 32),
         ones4, mskZ], axis=1)
    pkb = (p[:, None] >= p[None, :] + 1).astype(np.float16)
    return {"pka": pka, "pkb": pkb, "bwin": bwin}


def kernel(a_emb, b_emb, W1, W2_sum, W2_carry):
    global LAST_EXEC_TIME_NS, LAST_RESULT
    a_emb = np.asarray(a_emb, dtype=np.float32)
    b_emb = np.asarray(b_emb, dtype=np.float32)
    W1 = np.asarray(W1, dtype=np.float32)
    W2_sum = np.asarray(W2_sum, dtype=np.float32)
    W2_carry = np.asarray(W2_carry, dtype=np.float32)

    if not _structure_ok(W1, W2_sum, W2_carry):
        return _numpy_fallback(a_emb, b_emb, W1, W2_sum, W2_carry)

    from concourse.bass_utils import run_bass_kernel_spmd

    if "nc" not in _CACHE:
        _CACHE["nc"] = _build_nc()
    nc = _CACHE["nc"]

    in_map = _prep_inputs(a_emb, b_emb)
    trace = os.environ.get("KERNEL_TRACE", "") == "1"
    res = run_bass_kernel_spmd(nc, [in_map], [0], trace=trace)
    LAST_EXEC_TIME_NS = res.exec_time_ns
    LAST_RESULT = res
    return np.asarray(res.results[0]["out"]).astype(np.float32)


# revision 66
# speedup vs baseline: 1.1657x; 1.1657x over previous
"""Trainium2 Bass kernel for nn_C4ByteTransformer (4-step carry-propagation
softmax table lookup).

Contract: kernel(**inputs) takes FULL inputs (a_emb[4,256], b_emb[4,256],
W1[514,131072], W2_sum[131072,256], W2_carry[131072,2]) and returns the full
[4,256] float32 output.

Math: the tables are the canonical byte-add lookup structure (verified
exactly on host, with a numpy fallback otherwise):
  scores_i[k] = a_emb[i, a] + b_emb[i, b] + carry[c],  k = 512a + 2b + c
  weights = softmax(10*(scores - 2.5));  out_i = weights @ W2_sum;
  carry' = weights @ W2_carry,  W2_sum[k, (a+b+c) & 255] = 1,
  W2_carry[k, a+b+c >= 256] = 1.
Because exp is multiplicative over the separable score, with
EA[a] = exp(10 a_emb[i,a]), EB[b] = exp(10 b_emb[i,b]) and
s = sigmoid(20 carry_1 - 10) (= F1/(F0+F1)):
  out_i[m] = ((1-s) cyc[m] + s cyc[(m-1) mod 256]) / (ZA ZB)
  carry'_1 = (U + V s) / (ZA ZB)
where cyc = 256-point cyclic convolution of EA and EB,
U = sum_{a+b>=256} EA[a]EB[b], V = sum_{a+b=255} EA[a]EB[b].
The 131072-entry table never has to be touched.

V2 latency structure (the kernel is launch-overhead dominated; ~13.1us of
the exec time is fixed preamble/DMA-latency/teardown measured with a
trivial kernel):
 - The carry recursion is rewritten in tanh form:
     t_{i+1} = tanh(5 vz_i t_i + (10 uz_i + 5 vz_i - 5)),  s = (1+t)/2,
   which the ACT engine evaluates as ONE Tanh per step with per-partition
   scale/bias APs. Tanh lives in the same activation-function set as Exp
   (exp_and_others), so no table swap and no DVE round-trips: the whole
   chain is 3 back-to-back ACT ops.
 - U/V/Z sums: one [128x128] triangular matmul (tri rides the pk DMA as
   f32, bitcast to f32r) gives within-half suffix sums for both halves at
   once; element products and folds are split across DVE and GpSimd; one
   ones-lhsT matmul pair reduces partitions and broadcasts to partitions
   0-3.
 - Final combine: out = zsel*(cyc + ssel*(rot(cyc)-cyc)). d = rot-cyc and
   the zsel pre-scales run on DVE while the chain finishes, leaving a
   single [4,256] op after the step-select.
 - DMA: pkA (small, gates everything) on the sync queue, pkB (a8m+tri) on
   the tensor queue, the four Hankel windows split over gpsimd+vector
   queues. Constants (ones, diag mask) ride pkA; one activation-table
   load; 5 input DMA instructions total.
"""

import os

import numpy as np

NSTEP = 4
D = 256
NE = 131072

_CACHE = {}

LAST_EXEC_TIME_NS = None
LAST_RESULT = None

T0 = float(np.tanh(-5.0))  # chain state for step 0 (s0 = sigmoid(-10))


def _build_nc():
    import concourse.bacc as bacc
    import concourse.mybir as mybir
    import concourse.tile as tile

    f32 = mybir.dt.float32
    f32r = mybir.dt.float32r
    f16 = mybir.dt.float16
    mult = mybir.AluOpType.mult
    add = mybir.AluOpType.add
    subtract = mybir.AluOpType.subtract
    Exp = mybir.ActivationFunctionType.Exp
    Tanh = mybir.ActivationFunctionType.Tanh

    nc = bacc.Bacc("TRN2", target_bir_lowering=False, debug=False,
                   num_devices=1)

    # pkA [128, 57]: b8 (0:8, [bh,i]), a8 (8:16, [ah,i]),
    # a8m (16:48, [ah, i, i'] step-masked), ones4 (48:52),
    # mskZ = diag(1.0) rows 0-3 (52:56), plus a ones column (56) so the
    # ssel accumulate picks up its +0.5 offset for free. Cols 0:48 are
    # exp'd by ONE ACT into f16 (exp(10x) <= e^10 fits; masked -5 lanes
    # underflow to 0).
    pka = nc.dram_tensor("pka", [128, 57], f32, kind="ExternalInput")
    # pkB: tri [128, 128] in f16 (0/1 exact) for a 1-cycle/row suffix
    # matmul with the f16 exp tile as rhs.
    pkb = nc.dram_tensor("pkb", [128, 128], f16, kind="ExternalInput")
    # fp16 halves the window DMA packet time; exp(10b) quantization error
    # from fp16 b-values is ~2.4e-3 worst-case, well inside tolerance.
    # Shaped [.., 2, 128] so the ah1 matmul can take the half-rotated
    # period as ONE reversed-block rhs view.
    bwin = nc.dram_tensor("bwin", [128, NSTEP, 2, 128], f16,
                          kind="ExternalInput")
    # fp16 output: DVE 2-byte ops run in 2x mode and the values are
    # normalized (~[0,1]) before quantization; host casts back to f32.
    out = nc.dram_tensor("out", [NSTEP, D], f16, kind="ExternalOutput")

    with tile.TileContext(nc) as tc:
        with (
            tc.tile_pool(name="sb", bufs=1) as sb,
            tc.tile_pool(name="small", bufs=1) as small,
            tc.tile_pool(name="psA", bufs=1, space="PSUM") as psA,
            tc.tile_pool(name="psC", bufs=1, space="PSUM") as psC,
            tc.tile_pool(name="psD", bufs=1, space="PSUM") as psD,
        ):
            # ---- input DMAs, spread across queues ----
            pka_sb = sb.tile([128, 57], f32)
            nc.sync.dma_start(pka_sb[:], pka[:])
            pkb_sb = sb.tile([128, 128], f16)
            nc.scalar.dma_start(pkb_sb[:], pkb[:])
            bwin_sb = sb.tile([128, NSTEP, 2, 128], f16)
            # windows 0/1 on the gpsimd queue, 2/3 on sync behind pkA:
            # win0 keeps its packet-bandwidth head start (it gates the
            # serial exp train), and two queues halve descriptor time.
            nc.gpsimd.dma_start(bwin_sb[:, 0:1, :, :], bwin[:, 0:1, :, :])
            nc.sync.dma_start(bwin_sb[:, 2:3, :, :], bwin[:, 2:3, :, :])
            nc.gpsimd.dma_start(bwin_sb[:, 1:2, :, :], bwin[:, 1:2, :, :])
            nc.sync.dma_start(bwin_sb[:, 3:4, :, :], bwin[:, 3:4, :, :])

            # ---- one exp for b8+a8+a8m (ACT, exp_and_others table) ----
            # f16 out: PE runs 16-bit matmuls at 1 cycle/row (f32r needs 4
            # at these sizes); exp(10*a|b) <= e^10 fits fp16, the -5-masked
            # lanes underflow to exactly 0, and products reach e^20 only in
            # f32 outputs/PSUM.
            epk = sb.tile([128, 48], f16)
            nc.scalar.activation(epk[:], pka_sb[:, 0:48], Exp, scale=10.0)
            ewin = sb.tile([128, NSTEP, 2, 128], f16)
            for i in range(NSTEP):
                nc.scalar.activation(ewin[:, i:i + 1, :, :],
                                     bwin_sb[:, i:i + 1, :, :], Exp,
                                     scale=10.0)

            def ebv(bh):  # exp(b8)[:, bh, :] (f16)
                return epk[:, 4 * bh:4 * bh + 4]

            def eav(ah):  # exp(a8)[:, ah, :] (f16)
                return epk[:, 8 + 4 * ah:12 + 4 * ah]

            def eamv(ah, i):  # masked exp(a8) lhsT block, f16
                o = 16 + 16 * ah + 4 * i
                return epk[:, o:o + 4]

            ones4 = pka_sb[:, 48:52]
            mskZ = pka_sb[0:4, 52:56]
            mskZx = pka_sb[0:4, 52:57]
            ones16 = sb.tile([128, 4], f16)
            nc.vector.memset(ones16[:], 1.0)
            triv = pkb_sb[:]

            # ---- suffix sums, both halves in one matmul ----
            # suf[p, (bh,i)] = sum_{q>p} EB_i[128*bh + q]
            suf_ps = psA.tile([128, 2, NSTEP], f32)
            nc.tensor.matmul(suf_ps[:].opt(), lhsT=triv, rhs=epk[:, 0:8],
                             start=True, stop=True)

            # ---- element products (all DVE; v-products first, they only
            # need epka while the u-products wait on the suffix matmul) ----
            # scr: (u1, v1, u2, v2) so the fold can add halves [0:2]+[2:4].
            scr = sb.tile([128, 4, NSTEP], f32)
            nc.vector.tensor_tensor(out=scr[:, 1, :], in0=eav(0),
                                    in1=ebv(1), op=mult)
            nc.vector.tensor_tensor(out=scr[:, 3, :], in0=eav(1),
                                    in1=ebv(0), op=mult)
            nc.vector.tensor_tensor(out=scr[:, 0, :], in0=eav(0),
                                    in1=suf_ps[:, 1, :], op=mult)
            nc.vector.tensor_tensor(out=scr[:, 2, :], in0=eav(1),
                                    in1=suf_ps[:, 0, :], op=mult)

            # ---- partition reduction + broadcast to partitions 0-3 ----
            # red[p, 0, h, k, i]: (h, k) = (u1, v1 | u2, v2)
            # red[p, 1, k, h, i]: (k, h) = (zb0, zb1 | za0, za1)
            red_ps = psC.tile([NSTEP, 2, 2, 2, NSTEP], f32)
            nc.tensor.matmul(red_ps[:, 0, :, :, :].opt(), lhsT=ones4,
                             rhs=scr[:].opt(), start=True, stop=True)
            red_mm2 = nc.tensor.matmul(red_ps[:, 1, :, :, :].opt(),
                                       lhsT=ones16[:], rhs=epk[:, 0:16],
                                       start=True, stop=True)

            # ---- folds (PSUM allows only one PSUM operand per op: copy
            # the 4x32 reduction block to SBUF once, fold from there) ----
            red_sb = small.tile([NSTEP, 2, 2, 2, NSTEP], f32)
            nc.vector.tensor_copy(out=red_sb[:], in_=red_ps[:])
            # sums1 = (U', V); U = U' + ZA1*ZB1 (cross-half correction).
            sums1 = small.tile([NSTEP, 2, NSTEP], f32)
            nc.vector.tensor_tensor(out=sums1[:], in0=red_sb[:, 0, 0, :, :],
                                    in1=red_sb[:, 0, 1, :, :], op=add)
            V5 = small.tile([NSTEP, NSTEP], f32)
            nc.vector.tensor_scalar(out=V5[:], in0=sums1[:, 1, :],
                                    scalar1=5.0, scalar2=None, op0=mult)
            tzz = small.tile([NSTEP, NSTEP], f32)
            nc.gpsimd.tensor_tensor(out=tzz[:], in0=red_sb[:, 1, 0, 1, :],
                                    in1=red_sb[:, 1, 1, 1, :], op=mult)
            U = small.tile([NSTEP, NSTEP], f32)
            nc.vector.tensor_tensor(out=U[:], in0=sums1[:, 0, :], in1=tzz[:],
                                    op=add)
            # W1 = 10U + 5V (DVE) while GpSimd folds Z = (ZB0+ZB1)(ZA0+ZA1)
            W1 = small.tile([NSTEP, NSTEP], f32)
            nc.vector.scalar_tensor_tensor(out=W1[:], in0=U[:], scalar=10.0,
                                           in1=V5[:], op0=mult, op1=add)
            sums2 = small.tile([NSTEP, 2, NSTEP], f32)
            nc.gpsimd.tensor_tensor(out=sums2[:], in0=red_sb[:, 1, :, 0, :],
                                    in1=red_sb[:, 1, :, 1, :], op=add)
            Z = small.tile([NSTEP, NSTEP], f32)
            nc.gpsimd.tensor_tensor(out=Z[:], in0=sums2[:, 0, :],
                                    in1=sums2[:, 1, :], op=mult)

            zbi = small.tile([NSTEP, NSTEP], f32)
            nc.vector.reciprocal(zbi[:], Z[:])

            # ---- chain scale/bias ----
            # scale_i = 5 V_i zbi_i;  bias_i = (10U + 5V)_i zbi_i - 5
            bW = small.tile([NSTEP, NSTEP], f32)
            nc.vector.tensor_tensor(out=bW[:], in0=W1[:], in1=zbi[:],
                                    op=mult)
            scale = small.tile([NSTEP, NSTEP], f32)
            nc.vector.tensor_tensor(out=scale[:], in0=V5[:], in1=zbi[:],
                                    op=mult)
            bias = small.tile([NSTEP, NSTEP], f32)
            bias_ts = nc.vector.tensor_scalar(out=bias[:], in0=bW[:],
                                              scalar1=-5.0, scalar2=None,
                                              op0=add)

            # zsel[p] = zbi[p, p]: fused mask-mult + free-reduce in one STT.
            # Pinned after bias so the scheduler can't slot it into the
            # recip->bias stretch of the chain-critical DVE queue.
            zm = small.tile([NSTEP, NSTEP], f32)
            zsel = small.tile([NSTEP, 1], f32)
            zsel_stt = nc.vector.scalar_tensor_tensor(
                out=zm[:], in0=zbi[:], scalar=1.0, in1=mskZ, op0=mult,
                op1=mult, accum_out=zsel[:])
            tile.add_dep_helper(zsel_stt.ins, bias_ts.ins, False,
                                "chain bias before zsel")

            # ---- carry chain: 3 back-to-back Tanh ACTs ----
            # T col 4 = 1.0 feeds the +0.5 into the ssel accumulate.
            T = small.tile([NSTEP, NSTEP + 1], f32)
            nc.vector.memset(T[:, 0:1], T0)
            nc.vector.memset(T[:, 4:5], 1.0)
            for i in range(NSTEP - 1):
                nc.scalar.activation(T[:, i + 1:i + 2], T[:, i:i + 1], Tanh,
                                     bias=bias[:, i:i + 1],
                                     scale=scale[:, i:i + 1])

            # ---- convolutions: 12 matmuls accumulate into prt[i, m] ----
            # (three split MMs per step pipeline better on the PE than two
            # fused 256-col ones)
            prt = psD.tile([NSTEP, 256], f32)
            for i in range(NSTEP):
                cmm = nc.tensor.matmul(prt[:], lhsT=eamv(0, i),
                                       rhs=ewin[:, i, :, :].opt(),
                                       start=(i == 0), stop=False)
                if i == 0:
                    # keep the bias-path reduction matmuls ahead of the
                    # convolutions on the PE queue
                    tile.add_dep_helper(cmm.ins, red_mm2.ins, False,
                                        "red MMs before convs")
                nc.tensor.matmul(prt[:, 0:128], lhsT=eamv(1, i),
                                 rhs=ewin[:, i, 1, :], start=False,
                                 stop=False)
                nc.tensor.matmul(prt[:, 128:256], lhsT=eamv(1, i),
                                 rhs=ewin[:, i, 0, :], start=False,
                                 stop=(i == NSTEP - 1))

            # ---- combine: out = zsel*cyc + ssel*(zsel*rot(cyc)-zsel*cyc)
            # fp16 pre2/dz/comb: values are normalized by zsel (~[0,1]),
            # and all-2-byte SBUF operands put the DVE in its 2x mode.
            # pre2 = [zsel*cyc[255] | zsel*cyc], so rot(cyc)*zsel is its
            # 0:256 view and dz is ONE self-shifted subtract (wrap incl.)
            pre2 = sb.tile([NSTEP, 257], f16)
            nc.vector.tensor_scalar(out=pre2[:, 0:1], in0=prt[:, 255:256],
                                    scalar1=zsel[:], scalar2=None, op0=mult)
            nc.vector.tensor_scalar(out=pre2[:, 1:257], in0=prt[:],
                                    scalar1=zsel[:], scalar2=None, op0=mult)
            dz = sb.tile([NSTEP, 256], f16)
            dz_tt = nc.vector.tensor_tensor(out=dz[:], in0=pre2[:, 0:256],
                                            in1=pre2[:, 1:257], op=subtract)
            # ssel[p] = (1 + T[p, p]) / 2 in ONE fused accumulate:
            # sum((0.5*[T|1]) * [diag|1]) = 0.5*T[p,p] + 0.5. Pinned after
            # dz so the scheduler can't hoist the chain-gated select ahead
            # of the big pre2/dz ops on the DVE queue.
            tm = small.tile([NSTEP, NSTEP + 1], f32)
            ssel = small.tile([NSTEP, 1], f32)
            ssel_stt = nc.vector.scalar_tensor_tensor(
                out=tm[:], in0=T[:], scalar=0.5, in1=mskZx, op0=mult,
                op1=mult, accum_out=ssel[:])
            tile.add_dep_helper(ssel_stt.ins, dz_tt.ins, False,
                                "big combine ops before chain select")
            comb = sb.tile([NSTEP, D], f16)
            nc.vector.scalar_tensor_tensor(out=comb[:], in0=dz[:],
                                           scalar=ssel[:],
                                           in1=pre2[:, 1:257],
                                           op0=mult, op1=add)
            nc.sync.dma_start(out[:], comb[:])

    nc.compile()
    return nc


def _structure_ok(W1, W2_sum, W2_carry):
    """Exact check that the tables are the canonical byte-add structure."""
    k = np.arange(NE)
    a = k >> 9
    b = (k >> 1) & 255
    c = k & 1
    total = a + b + c
    if W1.shape != (514, NE) or W2_sum.shape != (NE, D):
        return False
    if W2_carry.shape != (NE, 2):
        return False
    if not (W1[a, k] == 1.0).all():
        return False
    if not (W1[256 + b, k] == 1.0).all():
        return False
    if not (W1[512 + c, k] == 1.0).all():
        return False
    if np.abs(W1).sum(dtype=np.float64) != 3.0 * NE:
        return False
    if not (W2_sum[k, total & 255] == 1.0).all():
        return False
    if np.abs(W2_sum).sum(dtype=np.float64) != float(NE):
        return False
    if not (W2_carry[k, (total >= 256).astype(np.int64)] == 1.0).all():
        return False
    if np.abs(W2_carry).sum(dtype=np.float64) != float(NE):
        return False
    return True


def _numpy_fallback(a_emb, b_emb, W1, W2_sum, W2_carry):
    carry = np.zeros(2, dtype=np.float64)
    carry[0] = 1.0
    outs = []
    W1 = W1.astype(np.float64)
    for i in range(NSTEP):
        x = np.concatenate([a_emb[i], b_emb[i], carry]).astype(np.float64)
        scores = x @ W1
        z = (scores - 2.5) * 10.0
        z -= z.max()
        w = np.exp(z)
        w /= w.sum()
        outs.append(w @ W2_sum.astype(np.float64))
        carry = w @ W2_carry.astype(np.float64)
    return np.stack(outs).astype(np.float32)


def _prep_inputs(a_emb, b_emb):
    p = np.arange(128)
    # bwin[j, i, x] = b_emb[i, (j + x + 129) mod 256], one cyclic period
    b_ext = np.take(b_emb, (np.arange(383) + 129) % 256, axis=1)
    bwin = np.ascontiguousarray(
        np.lib.stride_tricks.sliding_window_view(b_ext, 256, axis=1)
        .transpose(1, 0, 2)
    ).astype(np.float16).reshape(128, NSTEP, 2, 128)
    # a8[p, ah, i] = a_emb[i, 128 ah + 127 - p]
    a_r = a_emb[:, ::-1]
    a8 = np.ascontiguousarray(
        a_r.reshape(NSTEP, 2, 128)[:, ::-1, :].transpose(2, 1, 0)
    ).astype(np.float32)
    # a8m: step-masked copy (off-step columns -5 -> exp(10x) ~ 2e-22)
    a8m = np.full((128, 2, NSTEP, NSTEP), -5.0, dtype=np.float32)
    for i in range(NSTEP):
        a8m[:, :, i, i] = a8[:, :, i]
    # b8[p, bh, i] = b_emb[i, 128 bh + p]
    b8 = np.ascontiguousarray(
        b_emb.reshape(NSTEP, 2, 128).transpose(2, 1, 0)
    ).astype(np.float32)
    ones4 = np.ones((128, 4), dtype=np.float32)
    mskZ = np.zeros((128, 5), dtype=np.float32)
    mskZ[np.arange(4), np.arange(4)] = 1.0
    mskZ[:, 4] = 1.0  # +0.5 offset column for the ssel accumulate
    pka = np.concatenate(
        [b8.reshape(128, 8), a8.reshape(128, 8), a8m.reshape(128, 32),
         ones4, mskZ], axis=1)
    pkb = (p[:, None] >= p[None, :] + 1).astype(np.float16)
    return {"pka": pka, "pkb": pkb, "bwin": bwin}


def kernel(a_emb, b_emb, W1, W2_sum, W2_carry):
    global LAST_EXEC_TIME_NS, LAST_RESULT
    a_emb = np.asarray(a_emb, dtype=np.float32)
    b_emb = np.asarray(b_emb, dtype=np.float32)
    W1 = np.asarray(W1, dtype=np.float32)
    W2_sum = np.asarray(W2_sum, dtype=np.float32)
    W2_carry = np.asarray(W2_carry, dtype=np.float32)

    if not _structure_ok(W1, W2_sum, W2_carry):
        return _numpy_fallback(a_emb, b_emb, W1, W2_sum, W2_carry)

    from concourse.bass_utils import run_bass_kernel_spmd

    if "nc" not in _CACHE:
        _CACHE["nc"] = _build_nc()
    nc = _CACHE["nc"]

    in_map = _prep_inputs(a_emb, b_emb)
    trace = os.environ.get("KERNEL_TRACE", "") == "1"
    res = run_bass_kernel_spmd(nc, [in_map], [0], trace=trace)
    LAST_EXEC_TIME_NS = res.exec_time_ns
    LAST_RESULT = res
    return np.asarray(res.results[0]["out"]).astype(np.float32)
